# revision 14
# baseline (speedup 1.0000x reference)
"""v6 fallback (measured 176621 ns): 4 AGs (2x1MB x^T by quarter, 2x1MB V
fenced on the x AGs), host-pre-tiled partition-major DMAs, chunk-major S^T
scores, two-pass AV. See kernel.py for the full design commentary."""
import numpy as np
import ml_dtypes
from contextlib import ExitStack

import concourse.bass as bass
import concourse.tile as tile
import concourse.mybir as mybir
from concourse.bass_utils import run_bass_kernel_spmd
from concourse.masks import make_identity

F32 = mybir.dt.float32
BF16 = mybir.dt.bfloat16
AF = mybir.ActivationFunctionType
AX = mybir.AxisListType

B, S, E, D = 4, 2048, 1024, 1024
NCORES = 8
NSLOT = 8
NQ = NSLOT * 128
HT = S // 2
HH = HT // 2
EC = E // 128
NCH = S // 128
QW = EC * HH
SCALE = 1.0 / 32.0
MASKVAL = -30000.0
GROUPS = [[0, 1], [2, 3], [4, 5], [6, 7]]

_prog = None


def _kmin(c):
    return c // 2 + 1


def _split_multi_waits(nc, max_waits=1):
    n = 0
    for f in nc.m.functions:
        for b in f.blocks:
            insts = b.instructions
            out = []
            changed = False
            for ins in insts:
                si = ins.sync_info
                if si is not None and len(si.on_wait) > max_waits:
                    waits = list(si.on_wait)
                    for w in waits[:-max_waits]:
                        nop = mybir.InstNoOp(name=f"I-waitsplit-{n}")
                        n += 1
                        nop.engine = ins.engine
                        nop.sync_info = mybir.SyncInfo(on_wait=[w], on_update=[])
                        out.append(nop)
                    ins.sync_info = mybir.SyncInfo(
                        on_wait=waits[-max_waits:], on_update=list(si.on_update))
                    changed = True
                out.append(ins)
            if changed:
                b.instructions = out
    return nc


def _build(split=True):
    nc = bass.Bass(trn_type="TRN2", target_bir_lowering=False, debug=False)
    xo_in = [nc.dram_tensor(f"xo{g}", [128, QW], BF16,
                            kind="ExternalInput").ap() for g in range(2)]
    xq_in = nc.dram_tensor("xq", [128, EC * NQ], BF16, kind="ExternalInput").ap()
    m_in = nc.dram_tensor("m", [128, EC * E], BF16, kind="ExternalInput").ap()
    wv_in = nc.dram_tensor("wv", [128, EC * D], BF16, kind="ExternalInput").ap()
    maskin = nc.dram_tensor("maskT", [256, 128], BF16, kind="ExternalInput").ap()
    onesin = nc.dram_tensor("ones", [128, 1], BF16, kind="ExternalInput").ap()
    out = nc.dram_tensor("out", [NQ, D], F32, kind="ExternalOutput").ap()

    bncX, gathX = [], []
    for g in range(2):
        bncX.append(nc.dram_tensor(f"bncX{g}", [128, QW], BF16).ap())
        gathX.append(nc.dram_tensor(f"gathX{g}", [2, 128, QW], BF16).ap())
    bncV, gathV = [], []
    for v in range(2):
        bncV.append(nc.dram_tensor(f"bncV{v}", [128, 4 * D + 16], BF16).ap())
        gathV.append(nc.dram_tensor(f"gathV{v}", [2, 128, 4 * D + 16], BF16).ap())

    with tile.TileContext(nc) as tc, ExitStack() as ctx:
        for g in range(2):
            nc.scalar.dma_start(bncX[g][:], xo_in[g][:])
            nc.gpsimd.collective_compute(
                "AllGather", mybir.AluOpType.bypass, replica_groups=GROUPS,
                ins=[bncX[g].opt()], outs=[gathX[g].opt()])

        const = ctx.enter_context(tc.tile_pool(name="const", bufs=1))
        ident = const.tile([128, 128], BF16)
        make_identity(nc, ident[:])
        maskT = const.tile([128, 256], BF16)
        nc.scalar.dma_start(maskT[:, 0:128], maskin[0:128, :])
        nc.scalar.dma_start(maskT[:, 128:256], maskin[128:256, :])
        ones = const.tile([128, 1], BF16)
        nc.scalar.dma_start(ones[:], onesin[:])

        qtp = ctx.enter_context(tc.tile_pool(name="qtp", bufs=1))
        zts = qtp.tile([128, EC * NQ], BF16, name="zts")

        with tc.tile_pool(name="wp", bufs=1) as wp, \
             tc.tile_pool(name="xp", bufs=1) as xp, \
             tc.tile_pool(name="st", bufs=1) as stp, \
             tc.tile_pool(name="ps1", bufs=4, space="PSUM") as pp:
            wv = wp.tile([128, EC * D], BF16, name="wv")
            m = wp.tile([128, EC * E], BF16, name="m")
            xo = xp.tile([128, 2 * QW], BF16, name="xo")
            xq = xp.tile([128, EC * NQ], BF16, name="xq")

            half = EC * D // 2
            nc.sync.dma_start(xo[:, 0:QW // 2], xo_in[0][:, 0:QW // 2])
            # wv is host-tiled h-major: the first 1MB piece covers every
            # e-chunk of the h=0 output half, so the first V psum group
            # only needs 1.5MB of loads instead of 2.5MB
            nc.sync.dma_start(wv[:, 0:half], wv_in[:, 0:half])
            nc.sync.dma_start(xo[:, QW // 2:QW], xo_in[0][:, QW // 2:QW])
            nc.sync.dma_start(wv[:, half:], wv_in[:, half:])
            nc.sync.dma_start(xo[:, QW:2 * QW], xo_in[1][:])
            nc.sync.dma_start(m[:, 0:half], m_in[:, 0:half])
            nc.sync.dma_start(m[:, half:], m_in[:, half:])
            nc.sync.dma_start(xq[:, 0:half], xq_in[:, 0:half])
            nc.sync.dma_start(xq[:, half:], xq_in[:, half:])

            vown = stp.tile([128, (HT // 128) * D], BF16, name="vown")
            for v in range(2):
                for tl in range(HH // 128):
                    t = v * (HH // 128) + tl
                    xcol = v * QW + tl * 128
                    for h in range(2):
                        ps = pp.tile([128, 512], F32, name=f"pv{t}_{h}", tag="pp")
                        for e in range(EC):
                            nc.tensor.matmul(
                                ps[:],
                                xo[:, xcol + e * HH:xcol + e * HH + 128],
                                wv[:, h * 4096 + e * 512:h * 4096 + (e + 1) * 512],
                                start=(e == 0), stop=(e == EC - 1))
                        nc.vector.tensor_copy(
                            vown[:, t * D + h * 512:t * D + (h + 1) * 512],
                            ps[:])
                nc.scalar.dma_start(bncV[v][:, 0:4 * D],
                                    vown[:, v * 4 * D:(v + 1) * 4 * D])
                if v == 0:
                    # fence on sync: on scalar it would head-of-line block
                    # the gather loads behind it until the x AG completes
                    nc.sync.dma_start(bncV[0][0:1, 4 * D:4 * D + 16],
                                      gathX[0][1, 0:1, 0:16])
                    nc.gpsimd.collective_compute(
                        "AllGather", mybir.AluOpType.bypass,
                        replica_groups=GROUPS,
                        ins=[bncV[0].opt()], outs=[gathV[0].opt()])

            for d in range(EC):
                for g in range(2):
                    ps = pp.tile([128, 512], F32, name=f"pq{d}_{g}", tag="pp")
                    for e in range(EC):
                        nc.tensor.matmul(
                            ps[:],
                            m[:, e * E + d * 128:e * E + (d + 1) * 128],
                            xq[:, e * NQ + g * 512:e * NQ + (g + 1) * 512],
                            start=(e == 0), stop=(e == EC - 1))
                    nc.vector.tensor_copy(
                        zts[:, d * NQ + g * 512:d * NQ + (g + 1) * 512], ps[:])

        kvp = ctx.enter_context(tc.tile_pool(name="kvp", bufs=1))
        xts = kvp.tile([128, 4 * QW], BF16, name="xts")
        vts = kvp.tile([128, NCH * D], BF16, name="vts")
        for g in range(2):
            for r in range(2):
                q4 = r * 2 + g
                eng = nc.sync if r == 0 else nc.scalar
                eng.dma_start(xts[:, q4 * QW:(q4 + 1) * QW], gathX[g][r])
        # V-b fence + AllGather, emitted after the xts loads so its
        # X1-gated fence DMA cannot head-of-line block them
        nc.sync.dma_start(bncV[1][0:1, 4 * D:4 * D + 16],
                          gathX[1][1, 0:1, 0:16])
        nc.gpsimd.collective_compute(
            "AllGather", mybir.AluOpType.bypass, replica_groups=GROUPS,
            ins=[bncV[1].opt()], outs=[gathV[1].opt()])
        for v in range(2):
            for r in range(2):
                t0 = r * 8 + v * 4
                eng = nc.sync if r == 0 else nc.scalar
                eng.dma_start(vts[:, t0 * D:(t0 + 4) * D],
                              gathV[v][r, :, 0:4 * D])

        def xtc(c, e):
            return (c // 4) * QW + e * HH + (c % 4) * 128

        att = ctx.enter_context(tc.tile_pool(name="att", bufs=1))
        stats = ctx.enter_context(tc.tile_pool(name="stats", bufs=1))
        linv = stats.tile([128, NSLOT], F32, name="linv")
        pT = {c: att.tile([128, 128 * (NSLOT + 1 - _kmin(c))], BF16,
                          name=f"pT{c}") for c in range(NCH)}
        osb = {k: att.tile([128, D], F32, name=f"osb{k}")
               for k in range(3, NSLOT + 1)}
        av_a = {k: [c for c in range(2 * k) if c % 8 < 4]
                for k in range(1, NSLOT + 1)}
        av_b = {k: [c for c in range(2 * k) if c % 8 >= 4]
                for k in range(1, NSLOT + 1)}

        with tc.tile_pool(name="ps3", bufs=1, space="PSUM") as pp3:
            ls = pp3.tile([128, 2], F32, name="ls", tag="lsp", bufs=1)

            def emit_lsum(k):
                for ci, c in enumerate(range(2 * k)):
                    j = k - _kmin(c)
                    nc.tensor.matmul(ls[:, 0:1],
                                     pT[c][:, j * 128:(j + 1) * 128],
                                     ones[:], start=(ci == 0),
                                     stop=(ci == 2 * k - 1))
                nc.vector.reciprocal(linv[:, k - 1:k], ls[:, 0:1])

            # X0 carries global chunks {0-3, 8-11}; process those first so
            # the PE never waits on the later X1 AllGather
            done = set()
            pending = set(range(1, NSLOT + 1))
            for c in [0, 1, 2, 3, 8, 9, 10, 11, 4, 5, 6, 7, 12, 13, 14, 15]:
                km = _kmin(c)
                w = 128 * (NSLOT + 1 - km)
                npc = (w + 511) // 512
                sT = [pp3.tile([128, 512], F32, name=f"sT{c}_{i}", tag="sps",
                               bufs=3) for i in range(npc)]
                for i in range(npc):
                    pw = min(512, w - i * 512)
                    qoff = (km - 1) * 128 + i * 512
                    msk = (i == 0)
                    for e in range(EC):
                        nc.tensor.matmul(
                            sT[i][:, :pw],
                            xts[:, xtc(c, e):xtc(c, e) + 128],
                            zts[:, e * NQ + qoff:e * NQ + qoff + pw],
                            start=(e == 0), stop=(e == EC - 1 and not msk))
                    if msk:
                        mo = 0 if c % 2 == 0 else 128
                        nc.tensor.matmul(sT[i][:, 0:128], ident[:],
                                         maskT[:, mo:mo + 128],
                                         start=False, stop=True,
                                         skip_group_check=True)
                    nc.scalar.activation(pT[c][:, i * 512:i * 512 + pw],
                                         sT[i][:, :pw], AF.Exp, scale=SCALE)
                # one-chunk-lag row-sum emission: a slot's ones-matmuls go
                # out only once every chunk it needs was emitted BEFORE the
                # current one, so the PE never waits on the exp it just fed
                for k in sorted(pending):
                    if all(cc in done for cc in range(2 * k)):
                        emit_lsum(k)
                        pending.discard(k)
                done.add(c)
            for k in sorted(pending):
                emit_lsum(k)

            for k in range(1, NSLOT + 1):
                ca = av_a[k]
                o_ps = [pp3.tile([128, 512], F32, name=f"oa{k}_{h}", tag="ops",
                                 bufs=4) for h in range(2)]
                for ci, c in enumerate(ca):
                    j = k - _kmin(c)
                    for h in range(2):
                        nc.tensor.matmul(o_ps[h][:],
                                         pT[c][:, j * 128:(j + 1) * 128],
                                         vts[:, c * D + h * 512:c * D + (h + 1) * 512],
                                         start=(ci == 0), stop=(ci == len(ca) - 1))
                if not av_b[k]:
                    o_fin = att.tile([128, D], F32, name=f"ofa{k}", tag="ofin",
                                     bufs=2)
                    for h in range(2):
                        nc.scalar.activation(o_fin[:, h * 512:(h + 1) * 512],
                                             o_ps[h][:], AF.Copy,
                                             scale=linv[:, k - 1:k])
                    nc.sync.dma_start(out[(k - 1) * 128:k * 128, :], o_fin[:])
                else:
                    for h in range(2):
                        nc.vector.tensor_scalar_mul(
                            osb[k][:, h * 512:(h + 1) * 512], o_ps[h][:],
                            linv[:, k - 1:k])

            for k in range(3, NSLOT + 1):
                cb = av_b[k]
                o_ps = [pp3.tile([128, 512], F32, name=f"ob{k}_{h}", tag="ops",
                                 bufs=4) for h in range(2)]
                for ci, c in enumerate(cb):
                    j = k - _kmin(c)
                    for h in range(2):
                        nc.tensor.matmul(o_ps[h][:],
                                         pT[c][:, j * 128:(j + 1) * 128],
                                         vts[:, c * D + h * 512:c * D + (h + 1) * 512],
                                         start=(ci == 0), stop=(ci == len(cb) - 1))
                o_fin = att.tile([128, D], F32, name=f"ofb{k}", tag="ofin",
                                 bufs=2)
                o_sc = att.tile([128, D], F32, name=f"osc{k}", tag="osc", bufs=2)
                for h in range(2):
                    hs = slice(h * 512, (h + 1) * 512)
                    nc.vector.tensor_scalar_mul(o_sc[:, hs], o_ps[h][:],
                                                linv[:, k - 1:k])
                    nc.vector.tensor_add(o_fin[:, hs], o_sc[:, hs], osb[k][:, hs])
                nc.sync.dma_start(out[(k - 1) * 128:k * 128, :], o_fin[:])
    if split:
        _split_multi_waits(nc)
    return nc


def _masks():
    j = np.arange(256)[:, None]
    i = np.arange(128)[None, :]
    bf = ml_dtypes.bfloat16
    maskT0 = np.where(j <= i, 0.0, MASKVAL).astype(bf)
    maskT1 = np.where(j <= 128 + i, 0.0, MASKVAL).astype(bf)
    return maskT0, maskT1


def _ptile(a):
    Erows, W = a.shape
    ec = Erows // 128
    return np.ascontiguousarray(
        a.reshape(ec, 128, W).transpose(1, 0, 2).reshape(128, ec * W))


def _in_maps(x, w_q, w_k, w_v):
    bf = ml_dtypes.bfloat16
    x = np.asarray(x, np.float32)
    m = (np.asarray(w_q, np.float32).T @ np.asarray(w_k, np.float32))
    m_t = _ptile(m.astype(bf))
    wv_t = _ptile(np.ascontiguousarray(np.asarray(w_v, np.float32).T).astype(bf))
    # h-major retile: col h*4096 + e*512 + c  (h = output 512-col half)
    wv_t = np.ascontiguousarray(
        wv_t.reshape(128, EC, 2, 512).transpose(0, 2, 1, 3).reshape(128, EC * D))
    maskT0, maskT1 = _masks()
    ones = np.ones((128, 1), dtype=bf)

    in_maps = []
    for c in range(NCORES):
        b, p = divmod(c, 2)
        xb = x[b]
        xoT = np.ascontiguousarray(xb[p * HT:(p + 1) * HT, :].T).astype(bf)
        xo0 = _ptile(np.ascontiguousarray(xoT[:, 0:HH]))
        xo1 = _ptile(np.ascontiguousarray(xoT[:, HH:HT]))
        qrows = np.concatenate(
            [xb[128 * (2 * (k - 1) + p):128 * (2 * (k - 1) + p) + 128, :]
             for k in range(1, NSLOT + 1)], axis=0)
        xq_t = _ptile(np.ascontiguousarray(qrows.T).astype(bf))
        in_maps.append({
            "xo0": xo0, "xo1": xo1, "xq": xq_t,
            "m": m_t, "wv": wv_t,
            "maskT": maskT0 if p == 0 else maskT1,
            "ones": ones,
        })
    return in_maps


def _scatter(per_core_out):
    out = np.empty((B, S, D), dtype=np.float32)
    for c in range(NCORES):
        b, p = divmod(c, 2)
        oc = per_core_out[c]
        for k in range(1, NSLOT + 1):
            g = 2 * (k - 1) + p
            out[b, 128 * g:128 * (g + 1), :] = oc[128 * (k - 1):128 * k, :]
    return out


def kernel(x, w_q, w_k, w_v):
    global _prog
    if _prog is None:
        _prog = _build()
    in_maps = _in_maps(x, w_q, w_k, w_v)
    res = run_bass_kernel_spmd(_prog, in_maps, list(range(NCORES)))
    return _scatter([res.results[c]["out"] for c in range(NCORES)])


# revision 15
# speedup vs baseline: 1.1032x; 1.1032x over previous
"""v6 fallback (measured 176621 ns): 4 AGs (2x1MB x^T by quarter, 2x1MB V
fenced on the x AGs), host-pre-tiled partition-major DMAs, chunk-major S^T
scores, two-pass AV. See kernel.py for the full design commentary."""
import numpy as np
import ml_dtypes
from contextlib import ExitStack

import concourse.bass as bass
import concourse.tile as tile
import concourse.mybir as mybir
from concourse.bass_utils import run_bass_kernel_spmd
from concourse.masks import make_identity

F32 = mybir.dt.float32
BF16 = mybir.dt.bfloat16
AF = mybir.ActivationFunctionType
AX = mybir.AxisListType

B, S, E, D = 4, 2048, 1024, 1024
NCORES = 8
NSLOT = 8
NQ = NSLOT * 128
HT = S // 2
HH = HT // 2
EC = E // 128
NCH = S // 128
QW = EC * HH
SCALE = 1.0 / 32.0
MASKVAL = -30000.0
GROUPS = [[0, 1], [2, 3], [4, 5], [6, 7]]

_prog = None


def _kmin(c):
    return c // 2 + 1


def _split_multi_waits(nc, max_waits=1):
    n = 0
    for f in nc.m.functions:
        for b in f.blocks:
            insts = b.instructions
            out = []
            changed = False
            for ins in insts:
                si = ins.sync_info
                if si is not None and len(si.on_wait) > max_waits:
                    waits = list(si.on_wait)
                    for w in waits[:-max_waits]:
                        nop = mybir.InstNoOp(name=f"I-waitsplit-{n}")
                        n += 1
                        nop.engine = ins.engine
                        nop.sync_info = mybir.SyncInfo(on_wait=[w], on_update=[])
                        out.append(nop)
                    ins.sync_info = mybir.SyncInfo(
                        on_wait=waits[-max_waits:], on_update=list(si.on_update))
                    changed = True
                out.append(ins)
            if changed:
                b.instructions = out
    return nc


def _build(split=True):
    nc = bass.Bass(trn_type="TRN2", target_bir_lowering=False, debug=False)
    xo_in = [nc.dram_tensor(f"xo{g}", [128, QW], BF16,
                            kind="ExternalInput").ap() for g in range(2)]
    xq_in = nc.dram_tensor("xq", [128, EC * NQ], BF16, kind="ExternalInput").ap()
    m_in = nc.dram_tensor("m", [128, EC * E], BF16, kind="ExternalInput").ap()
    wv_in = nc.dram_tensor("wv", [128, EC * D], BF16, kind="ExternalInput").ap()
    maskin = nc.dram_tensor("maskT", [256, 128], BF16, kind="ExternalInput").ap()
    onesin = nc.dram_tensor("ones", [128, 1], BF16, kind="ExternalInput").ap()
    out = nc.dram_tensor("out", [NQ, D], F32, kind="ExternalOutput").ap()

    bncX, gathX = [], []
    for g in range(2):
        bncX.append(nc.dram_tensor(f"bncX{g}", [128, QW], BF16).ap())
        gathX.append(nc.dram_tensor(f"gathX{g}", [2, 128, QW], BF16).ap())
    bncV, gathV = [], []
    for v in range(2):
        bncV.append(nc.dram_tensor(f"bncV{v}", [128, 4 * D + 16], BF16).ap())
        gathV.append(nc.dram_tensor(f"gathV{v}", [2, 128, 4 * D + 16], BF16).ap())

    with tile.TileContext(nc) as tc, ExitStack() as ctx:
        for g in range(2):
            nc.scalar.dma_start(bncX[g][:], xo_in[g][:])
            nc.gpsimd.collective_compute(
                "AllGather", mybir.AluOpType.bypass, replica_groups=GROUPS,
                ins=[bncX[g].opt()], outs=[gathX[g].opt()])

        const = ctx.enter_context(tc.tile_pool(name="const", bufs=1))
        ident = const.tile([128, 128], BF16)
        make_identity(nc, ident[:])
        maskT = const.tile([128, 256], BF16)
        nc.scalar.dma_start(maskT[:, 0:128], maskin[0:128, :])
        nc.scalar.dma_start(maskT[:, 128:256], maskin[128:256, :])
        ones = const.tile([128, 1], BF16)
        nc.scalar.dma_start(ones[:], onesin[:])

        qtp = ctx.enter_context(tc.tile_pool(name="qtp", bufs=1))
        zts = qtp.tile([128, EC * NQ], BF16, name="zts")

        with tc.tile_pool(name="wp", bufs=1) as wp, \
             tc.tile_pool(name="xp", bufs=1) as xp, \
             tc.tile_pool(name="st", bufs=1) as stp, \
             tc.tile_pool(name="ps1", bufs=4, space="PSUM") as pp:
            wv = wp.tile([128, EC * D], BF16, name="wv")
            m = wp.tile([128, EC * E], BF16, name="m")
            xo = xp.tile([128, 2 * QW], BF16, name="xo")
            xq = xp.tile([128, EC * NQ], BF16, name="xq")

            half = EC * D // 2
            nc.sync.dma_start(xo[:, 0:QW // 2], xo_in[0][:, 0:QW // 2])
            nc.sync.dma_start(wv[:, 0:half], wv_in[:, 0:half])
            nc.sync.dma_start(xo[:, QW // 2:QW], xo_in[0][:, QW // 2:QW])
            nc.sync.dma_start(wv[:, half:], wv_in[:, half:])
            nc.sync.dma_start(xo[:, QW:2 * QW], xo_in[1][:])
            nc.sync.dma_start(m[:, 0:half], m_in[:, 0:half])
            nc.sync.dma_start(m[:, half:], m_in[:, half:])
            nc.sync.dma_start(xq[:, 0:half], xq_in[:, 0:half])
            nc.sync.dma_start(xq[:, half:], xq_in[:, half:])

            vown = stp.tile([128, (HT // 128) * D], BF16, name="vown")
            for v in range(2):
                for tl in range(HH // 128):
                    t = v * (HH // 128) + tl
                    xcol = v * QW + tl * 128
                    for h in range(2):
                        ps = pp.tile([128, 512], F32, name=f"pv{t}_{h}", tag="pp")
                        for e in range(EC):
                            nc.tensor.matmul(
                                ps[:],
                                xo[:, xcol + e * HH:xcol + e * HH + 128],
                                wv[:, e * D + h * 512:e * D + (h + 1) * 512],
                                start=(e == 0), stop=(e == EC - 1))
                        nc.vector.tensor_copy(
                            vown[:, t * D + h * 512:t * D + (h + 1) * 512],
                            ps[:])
                nc.scalar.dma_start(bncV[v][:, 0:4 * D],
                                    vown[:, v * 4 * D:(v + 1) * 4 * D])
                nc.scalar.dma_start(bncV[v][0:1, 4 * D:4 * D + 16],
                                    gathX[v][1, 0:1, 0:16])
                nc.gpsimd.collective_compute(
                    "AllGather", mybir.AluOpType.bypass, replica_groups=GROUPS,
                    ins=[bncV[v].opt()], outs=[gathV[v].opt()])

            for d in range(EC):
                for g in range(2):
                    ps = pp.tile([128, 512], F32, name=f"pq{d}_{g}", tag="pp")
                    for e in range(EC):
                        nc.tensor.matmul(
                            ps[:],
                            m[:, e * E + d * 128:e * E + (d + 1) * 128],
                            xq[:, e * NQ + g * 512:e * NQ + (g + 1) * 512],
                            start=(e == 0), stop=(e == EC - 1))
                    nc.vector.tensor_copy(
                        zts[:, d * NQ + g * 512:d * NQ + (g + 1) * 512], ps[:])

        kvp = ctx.enter_context(tc.tile_pool(name="kvp", bufs=1))
        xts = kvp.tile([128, 4 * QW], BF16, name="xts")
        vts = kvp.tile([128, NCH * D], BF16, name="vts")
        for g in range(2):
            for r in range(2):
                q4 = r * 2 + g
                nc.sync.dma_start(xts[:, q4 * QW:(q4 + 1) * QW], gathX[g][r])
        for v in range(2):
            for r in range(2):
                t0 = r * 8 + v * 4
                nc.sync.dma_start(vts[:, t0 * D:(t0 + 4) * D],
                                  gathV[v][r, :, 0:4 * D])

        def xtc(c, e):
            return (c // 4) * QW + e * HH + (c % 4) * 128

        att = ctx.enter_context(tc.tile_pool(name="att", bufs=1))
        stats = ctx.enter_context(tc.tile_pool(name="stats", bufs=1))
        linv = stats.tile([128, NSLOT], F32, name="linv")
        pT = {c: att.tile([128, 128 * (NSLOT + 1 - _kmin(c))], BF16,
                          name=f"pT{c}") for c in range(NCH)}
        osb = {k: att.tile([128, D], F32, name=f"osb{k}")
               for k in range(3, NSLOT + 1)}
        av_a = {k: [c for c in range(2 * k) if c % 8 < 4]
                for k in range(1, NSLOT + 1)}
        av_b = {k: [c for c in range(2 * k) if c % 8 >= 4]
                for k in range(1, NSLOT + 1)}

        with tc.tile_pool(name="ps3", bufs=1, space="PSUM") as pp3:
            ls = pp3.tile([128, 2], F32, name="ls", tag="lsp", bufs=1)

            def emit_lsum(k):
                for ci, c in enumerate(range(2 * k)):
                    j = k - _kmin(c)
                    nc.tensor.matmul(ls[:, 0:1],
                                     pT[c][:, j * 128:(j + 1) * 128],
                                     ones[:], start=(ci == 0),
                                     stop=(ci == 2 * k - 1))
                nc.vector.reciprocal(linv[:, k - 1:k], ls[:, 0:1])

            for c in range(NCH):
                km = _kmin(c)
                w = 128 * (NSLOT + 1 - km)
                npc = (w + 511) // 512
                sT = [pp3.tile([128, 512], F32, name=f"sT{c}_{i}", tag="sps",
                               bufs=3) for i in range(npc)]
                for i in range(npc):
                    pw = min(512, w - i * 512)
                    qoff = (km - 1) * 128 + i * 512
                    msk = (i == 0)
                    for e in range(EC):
                        nc.tensor.matmul(
                            sT[i][:, :pw],
                            xts[:, xtc(c, e):xtc(c, e) + 128],
                            zts[:, e * NQ + qoff:e * NQ + qoff + pw],
                            start=(e == 0), stop=(e == EC - 1 and not msk))
                    if msk:
                        mo = 0 if c % 2 == 0 else 128
                        nc.tensor.matmul(sT[i][:, 0:128], ident[:],
                                         maskT[:, mo:mo + 128],
                                         start=False, stop=True,
                                         skip_group_check=True)
                    nc.scalar.activation(pT[c][:, i * 512:i * 512 + pw],
                                         sT[i][:, :pw], AF.Exp, scale=SCALE)
                if c >= 2 and c % 2 == 0:
                    emit_lsum(c // 2)
            emit_lsum(NSLOT)

            for k in range(1, NSLOT + 1):
                ca = av_a[k]
                o_ps = [pp3.tile([128, 512], F32, name=f"oa{k}_{h}", tag="ops",
                                 bufs=4) for h in range(2)]
                for ci, c in enumerate(ca):
                    j = k - _kmin(c)
                    for h in range(2):
                        nc.tensor.matmul(o_ps[h][:],
                                         pT[c][:, j * 128:(j + 1) * 128],
                                         vts[:, c * D + h * 512:c * D + (h + 1) * 512],
                                         start=(ci == 0), stop=(ci == len(ca) - 1))
                if not av_b[k]:
                    o_fin = att.tile([128, D], F32, name=f"ofa{k}", tag="ofin",
                                     bufs=2)
                    for h in range(2):
                        nc.scalar.activation(o_fin[:, h * 512:(h + 1) * 512],
                                             o_ps[h][:], AF.Copy,
                                             scale=linv[:, k - 1:k])
                    nc.sync.dma_start(out[(k - 1) * 128:k * 128, :], o_fin[:])
                else:
                    for h in range(2):
                        nc.vector.tensor_scalar_mul(
                            osb[k][:, h * 512:(h + 1) * 512], o_ps[h][:],
                            linv[:, k - 1:k])

            for k in range(3, NSLOT + 1):
                cb = av_b[k]
                o_ps = [pp3.tile([128, 512], F32, name=f"ob{k}_{h}", tag="ops",
                                 bufs=4) for h in range(2)]
                for ci, c in enumerate(cb):
                    j = k - _kmin(c)
                    for h in range(2):
                        nc.tensor.matmul(o_ps[h][:],
                                         pT[c][:, j * 128:(j + 1) * 128],
                                         vts[:, c * D + h * 512:c * D + (h + 1) * 512],
                                         start=(ci == 0), stop=(ci == len(cb) - 1))
                o_fin = att.tile([128, D], F32, name=f"ofb{k}", tag="ofin",
                                 bufs=2)
                o_sc = att.tile([128, D], F32, name=f"osc{k}", tag="osc", bufs=2)
                for h in range(2):
                    hs = slice(h * 512, (h + 1) * 512)
                    nc.vector.tensor_scalar_mul(o_sc[:, hs], o_ps[h][:],
                                                linv[:, k - 1:k])
                    nc.vector.tensor_add(o_fin[:, hs], o_sc[:, hs], osb[k][:, hs])
                nc.sync.dma_start(out[(k - 1) * 128:k * 128, :], o_fin[:])
    if split:
        _split_multi_waits(nc)
    return nc


def _masks():
    j = np.arange(256)[:, None]
    i = np.arange(128)[None, :]
    bf = ml_dtypes.bfloat16
    maskT0 = np.where(j <= i, 0.0, MASKVAL).astype(bf)
    maskT1 = np.where(j <= 128 + i, 0.0, MASKVAL).astype(bf)
    return maskT0, maskT1


def _ptile(a):
    Erows, W = a.shape
    ec = Erows // 128
    return np.ascontiguousarray(
        a.reshape(ec, 128, W).transpose(1, 0, 2).reshape(128, ec * W))


def _in_maps(x, w_q, w_k, w_v):
    bf = ml_dtypes.bfloat16
    x = np.asarray(x, np.float32)
    m = (np.asarray(w_q, np.float32).T @ np.asarray(w_k, np.float32))
    m_t = _ptile(m.astype(bf))
    wv_t = _ptile(np.ascontiguousarray(np.asarray(w_v, np.float32).T).astype(bf))
    maskT0, maskT1 = _masks()
    ones = np.ones((128, 1), dtype=bf)

    in_maps = []
    for c in range(NCORES):
        b, p = divmod(c, 2)
        xb = x[b]
        xoT = np.ascontiguousarray(xb[p * HT:(p + 1) * HT, :].T).astype(bf)
        xo0 = _ptile(np.ascontiguousarray(xoT[:, 0:HH]))
        xo1 = _ptile(np.ascontiguousarray(xoT[:, HH:HT]))
        qrows = np.concatenate(
            [xb[128 * (2 * (k - 1) + p):128 * (2 * (k - 1) + p) + 128, :]
             for k in range(1, NSLOT + 1)], axis=0)
        xq_t = _ptile(np.ascontiguousarray(qrows.T).astype(bf))
        in_maps.append({
            "xo0": xo0, "xo1": xo1, "xq": xq_t,
            "m": m_t, "wv": wv_t,
            "maskT": maskT0 if p == 0 else maskT1,
            "ones": ones,
        })
    return in_maps


def _scatter(per_core_out):
    out = np.empty((B, S, D), dtype=np.float32)
    for c in range(NCORES):
        b, p = divmod(c, 2)
        oc = per_core_out[c]
        for k in range(1, NSLOT + 1):
            g = 2 * (k - 1) + p
            out[b, 128 * g:128 * (g + 1), :] = oc[128 * (k - 1):128 * k, :]
    return out


def kernel(x, w_q, w_k, w_v):
    global _prog
    if _prog is None:
        _prog = _build()
    in_maps = _in_maps(x, w_q, w_k, w_v)
    res = run_bass_kernel_spmd(_prog, in_maps, list(range(NCORES)))
    return _scatter([res.results[c]["out"] for c in range(NCORES)])
